# revision 34
# baseline (speedup 1.0000x reference)
"""ALiBi-2D bias-add kernel for 8 Trainium2 NeuronCores.

out[b,h,i,j] = attn_scores[b,h,i,j] - slopes[h] * dist(coords[b,i], coords[b,j])

Sharding: the 32 (b,h) slices are split 4-per-core across 8 cores (all four
heads on a core share the same batch b, so the pairwise-distance tile is
computed once per row-tile and reused for all 4 heads).

dist^2 is computed on the TensorEngine as a rank-3 matmul:
  dist2[i,j] = [-2x_i, -2y_i, 1] . [x_j, y_j, x_j^2+y_j^2] + (x_i^2+y_i^2)
with the per-row norm added as the per-partition bias of the ACT sqrt that
reads the matmul result straight out of PSUM. All coordinate values are small
integers, so every step is exact in fp32.
"""

import numpy as np

import concourse.bacc as bacc
import concourse.mybir as mybir
from concourse.bass_utils import run_bass_kernel_spmd
from concourse.tile import TileContext

B, H, T = 2, 16, 2048
P = 128
NT = T // P  # row tiles per core
NB = T // 512  # psum banks per row tile
NCORES = 8
CPB = NCORES // B  # cores per batch entry (4)
HPC = H // CPB  # heads per core (4)

F32 = mybir.dt.float32


def _build_nc():
    nc = bacc.Bacc(
        "TRN2", target_bir_lowering=False, debug=False, num_devices=NCORES
    )
    AF = mybir.ActivationFunctionType
    OP = mybir.AluOpType

    scores = nc.dram_tensor("scores", [HPC, T, T], F32, kind="ExternalInput")
    cmat = nc.dram_tensor("cmat", [3, T], F32, kind="ExternalInput")
    rmat = nc.dram_tensor("rmat", [3, T], F32, kind="ExternalInput")
    ni = nc.dram_tensor("ni", [P, NT], F32, kind="ExternalInput")
    nslope = nc.dram_tensor("nslope", [P, HPC], F32, kind="ExternalInput")
    out = nc.dram_tensor("out", [HPC, T, T], F32, kind="ExternalOutput")

    scores_r = scores[:].rearrange("h r c -> r h c")
    out_r = out[:].rearrange("h r c -> r h c")

    with TileContext(nc) as tc:
        with (
            tc.tile_pool(name="const", bufs=1) as cpool,
            tc.tile_pool(name="psum", bufs=2, space="PSUM") as ppool,
            tc.tile_pool(name="dist", bufs=2) as dist_pool,
            tc.tile_pool(name="sin", bufs=5) as sin_pool,
            tc.tile_pool(name="sout", bufs=4) as sout_pool,
        ):
            cmat_t = cpool.tile([3, T], F32)
            rmat_t = cpool.tile([3, T], F32)
            ni_t = cpool.tile([P, NT], F32)
            nslope_t = cpool.tile([P, HPC], F32)
            # const loads on the scalar ring: the sync ring then carries
            # nothing but score loads, streaming from instruction 0
            nc.scalar.dma_start(out=cmat_t[:], in_=cmat[:])
            nc.scalar.dma_start(out=rmat_t[:], in_=rmat[:])
            nc.scalar.dma_start(out=ni_t[:], in_=ni[:])
            nc.scalar.dma_start(out=nslope_t[:], in_=nslope[:])

            for t in range(NT):
                r0 = t * P
                ps = ppool.tile([P, T], F32)
                for j in range(NB):
                    nc.tensor.matmul(
                        ps[:, j * 512 : (j + 1) * 512],
                        cmat_t[:, r0 : r0 + P],
                        rmat_t[:, j * 512 : (j + 1) * 512],
                        start=True,
                        stop=True,
                    )
                dist = dist_pool.tile([P, T], F32)
                # dist = sqrt(psum + ||c_i||^2)
                nc.scalar.activation(
                    dist[:], ps[:], AF.Sqrt, bias=ni_t[:, t : t + 1]
                )
                for hp in range(HPC // 2):
                    h0 = hp * 2
                    s = sin_pool.tile([P, 2, T], F32)
                    nc.sync.dma_start(
                        out=s[:], in_=scores_r[r0 : r0 + P, h0 : h0 + 2, :]
                    )
                    o = sout_pool.tile([P, 2, T], F32)
                    for k in range(2):
                        # out = (dist * -slope_h) + scores
                        nc.vector.scalar_tensor_tensor(
                            out=o[:, k, :],
                            in0=dist[:],
                            scalar=nslope_t[:, h0 + k : h0 + k + 1],
                            in1=s[:, k, :],
                            op0=OP.mult,
                            op1=OP.add,
                        )
                    # stores on the scalar engine's HWDGE ring so a store
                    # waiting on compute can't head-of-line-block loads on
                    # the sync ring
                    nc.scalar.dma_start(
                        out=out_r[r0 : r0 + P, h0 : h0 + 2, :], in_=o[:]
                    )
    nc.compile()
    return nc


def _shard_inputs(attn_scores, coords_xy, slopes):
    coords = coords_xy.astype(np.float32)
    slopes = np.asarray(slopes, dtype=np.float32)
    in_maps = []
    for c in range(NCORES):
        b = c // CPB
        h0 = (c % CPB) * HPC
        x = coords[b, :, 0]
        y = coords[b, :, 1]
        n = x * x + y * y
        cm = np.stack([-2.0 * x, -2.0 * y, np.ones_like(x)])
        rm = np.stack([x, y, n])
        in_maps.append(
            {
                "scores": np.ascontiguousarray(attn_scores[b, h0 : h0 + HPC]),
                "cmat": np.ascontiguousarray(cm),
                "rmat": np.ascontiguousarray(rm),
                "ni": np.ascontiguousarray(n.reshape(NT, P).T),
                "nslope": np.ascontiguousarray(
                    np.broadcast_to(-slopes[h0 : h0 + HPC][None, :], (P, HPC))
                ),
            }
        )
    return in_maps


def _run(attn_scores, coords_xy, slopes, trace=False):
    attn_scores = np.asarray(attn_scores, dtype=np.float32)
    coords_xy = np.asarray(coords_xy)
    nc = _build_nc()
    in_maps = _shard_inputs(attn_scores, coords_xy, slopes)
    res = run_bass_kernel_spmd(nc, in_maps, core_ids=list(range(NCORES)), trace=trace)
    full = np.empty((B, H, T, T), dtype=np.float32)
    for c in range(NCORES):
        b = c // CPB
        h0 = (c % CPB) * HPC
        full[b, h0 : h0 + HPC] = res.results[c]["out"]
    return full, res


def kernel(attn_scores, coords_xy, slopes):
    full, _ = _run(attn_scores, coords_xy, slopes, trace=False)
    return full


# revision 35
# speedup vs baseline: 1.0104x; 1.0104x over previous
"""ALiBi-2D bias-add kernel for 8 Trainium2 NeuronCores.

out[b,h,i,j] = attn_scores[b,h,i,j] - slopes[h] * dist(coords[b,i], coords[b,j])

Sharding: the 32 (b,h) slices are split 4-per-core across 8 cores (all four
heads on a core share the same batch b, so the pairwise-distance tile is
computed once per row-tile and reused for all 4 heads).

dist^2 is computed on the TensorEngine as a rank-3 matmul:
  dist2[i,j] = [-2x_i, -2y_i, 1] . [x_j, y_j, x_j^2+y_j^2] + (x_i^2+y_i^2)
with the per-row norm added as the per-partition bias of the ACT sqrt that
reads the matmul result straight out of PSUM. All coordinate values are small
integers, so every step is exact in fp32.
"""

import numpy as np

import concourse.bacc as bacc
import concourse.mybir as mybir
from concourse.bass_utils import run_bass_kernel_spmd
from concourse.tile import TileContext

B, H, T = 2, 16, 2048
P = 128
NT = T // P  # row tiles per core
NB = T // 512  # psum banks per row tile
NCORES = 8
CPB = NCORES // B  # cores per batch entry (4)
HPC = H // CPB  # heads per core (4)

F32 = mybir.dt.float32


def _build_nc():
    nc = bacc.Bacc(
        "TRN2", target_bir_lowering=False, debug=False, num_devices=NCORES
    )
    AF = mybir.ActivationFunctionType
    OP = mybir.AluOpType

    scores = nc.dram_tensor("scores", [HPC, T, T], F32, kind="ExternalInput")
    cmat = nc.dram_tensor("cmat", [3, T], F32, kind="ExternalInput")
    rmat = nc.dram_tensor("rmat", [3, T], F32, kind="ExternalInput")
    ni = nc.dram_tensor("ni", [P, NT], F32, kind="ExternalInput")
    nslope = nc.dram_tensor("nslope", [P, HPC], F32, kind="ExternalInput")
    out = nc.dram_tensor("out", [HPC, T, T], F32, kind="ExternalOutput")

    scores_r = scores[:].rearrange("h r c -> r h c")
    out_r = out[:].rearrange("h r c -> r h c")

    with TileContext(nc) as tc:
        with (
            tc.tile_pool(name="const", bufs=1) as cpool,
            tc.tile_pool(name="psum", bufs=2, space="PSUM") as ppool,
            tc.tile_pool(name="dist", bufs=2) as dist_pool,
            tc.tile_pool(name="sin", bufs=6) as sin_pool,
            tc.tile_pool(name="sout", bufs=3) as sout_pool,
        ):
            cmat_t = cpool.tile([3, T], F32)
            rmat_t = cpool.tile([3, T], F32)
            ni_t = cpool.tile([P, NT], F32)
            nslope_t = cpool.tile([P, HPC], F32)
            # const loads on the scalar ring: the sync ring then carries
            # nothing but score loads, streaming from instruction 0
            nc.scalar.dma_start(out=cmat_t[:], in_=cmat[:])
            nc.scalar.dma_start(out=rmat_t[:], in_=rmat[:])
            nc.scalar.dma_start(out=ni_t[:], in_=ni[:])
            nc.scalar.dma_start(out=nslope_t[:], in_=nslope[:])

            for t in range(NT):
                r0 = t * P
                ps = ppool.tile([P, T], F32)
                for j in range(NB):
                    nc.tensor.matmul(
                        ps[:, j * 512 : (j + 1) * 512],
                        cmat_t[:, r0 : r0 + P],
                        rmat_t[:, j * 512 : (j + 1) * 512],
                        start=True,
                        stop=True,
                    )
                dist = dist_pool.tile([P, T], F32)
                # dist = sqrt(psum + ||c_i||^2)
                nc.scalar.activation(
                    dist[:], ps[:], AF.Sqrt, bias=ni_t[:, t : t + 1]
                )
                for hp in range(HPC // 2):
                    h0 = hp * 2
                    s = sin_pool.tile([P, 2, T], F32)
                    nc.sync.dma_start(
                        out=s[:], in_=scores_r[r0 : r0 + P, h0 : h0 + 2, :]
                    )
                    o = sout_pool.tile([P, 2, T], F32)
                    for k in range(2):
                        # out = (dist * -slope_h) + scores
                        nc.vector.scalar_tensor_tensor(
                            out=o[:, k, :],
                            in0=dist[:],
                            scalar=nslope_t[:, h0 + k : h0 + k + 1],
                            in1=s[:, k, :],
                            op0=OP.mult,
                            op1=OP.add,
                        )
                    # stores on the scalar engine's HWDGE ring so a store
                    # waiting on compute can't head-of-line-block loads on
                    # the sync ring
                    nc.scalar.dma_start(
                        out=out_r[r0 : r0 + P, h0 : h0 + 2, :], in_=o[:]
                    )
    nc.compile()
    return nc


def _shard_inputs(attn_scores, coords_xy, slopes):
    coords = coords_xy.astype(np.float32)
    slopes = np.asarray(slopes, dtype=np.float32)
    in_maps = []
    for c in range(NCORES):
        b = c // CPB
        h0 = (c % CPB) * HPC
        x = coords[b, :, 0]
        y = coords[b, :, 1]
        n = x * x + y * y
        cm = np.stack([-2.0 * x, -2.0 * y, np.ones_like(x)])
        rm = np.stack([x, y, n])
        in_maps.append(
            {
                "scores": np.ascontiguousarray(attn_scores[b, h0 : h0 + HPC]),
                "cmat": np.ascontiguousarray(cm),
                "rmat": np.ascontiguousarray(rm),
                "ni": np.ascontiguousarray(n.reshape(NT, P).T),
                "nslope": np.ascontiguousarray(
                    np.broadcast_to(-slopes[h0 : h0 + HPC][None, :], (P, HPC))
                ),
            }
        )
    return in_maps


def _run(attn_scores, coords_xy, slopes, trace=False):
    attn_scores = np.asarray(attn_scores, dtype=np.float32)
    coords_xy = np.asarray(coords_xy)
    nc = _build_nc()
    in_maps = _shard_inputs(attn_scores, coords_xy, slopes)
    res = run_bass_kernel_spmd(nc, in_maps, core_ids=list(range(NCORES)), trace=trace)
    full = np.empty((B, H, T, T), dtype=np.float32)
    for c in range(NCORES):
        b = c // CPB
        h0 = (c % CPB) * HPC
        full[b, h0 : h0 + HPC] = res.results[c]["out"]
    return full, res


def kernel(attn_scores, coords_xy, slopes):
    full, _ = _run(attn_scores, coords_xy, slopes, trace=False)
    return full
